# revision 38
# baseline (speedup 1.0000x reference)
"""MoE low-rank adapters (top-1 routing) Trainium2 kernel.

Math (reference):
  xf = x.reshape(N, D)                 N=8192, D=2048, E=8, R=64
  logits = xf @ Wg.T                   [N, E]
  prob = softmax(logits); gate = argmax(prob); prob_sel = max(prob)
  h = xf @ A[e].T for all e            [N, E*R]
  y = (h * onehot(gate)) @ Bwt         [N, D]
  y *= SCALING * prob_sel

Distribution: data-parallel over tokens, 8 cores x 1024 tokens.

Design (v8 + 3-bank y rotation):
- gating in f32r: 16-matmul chain per 512-token pair into one [8, 512]
  PSUM band (1 cyc/row at free>=256; ~13-bit mantissa keeps every
  argmax identical to fp32 on well-separated logits), then PE
  transposes to natural [tok, e] for the softmax/argmax epilogue.
- h and y in bf16 (full-rate moving streams + FWL weight loads); x is
  cast f32r->bf16 on ACT/DVE, split per k-chunk.
- One dma_start rides one DMA engine (~100-125GB/s), so every large
  transfer is chunked into 128-256KB pieces with >=2KB contiguous
  runs, fanned across the three DMA queues (sync/scalar HWDGE +
  gpsimd SWDGE); aggregate parallelism = outstanding pieces.
- DMA priority: pair-0 x pieces + adapter A first (PE goes dense
  right after warmup so the HAM clock ramps to 2.4GHz); pair-1 x and
  the B weights stream during pair-0 compute; the mask DRAM-broadcast
  rides a quiet queue. y drains per (t, o) [128, 512] chunk over all
  three queues to keep the tail short.
- PE order: warmup, g0, h0 (epilogue PE ops slotted between chunks so
  the mask broadcast lands while h streams), g1, h1, y0, y1.
"""

import sys

for _p in ("/opt/trn_rl_repo",):
    if _p not in sys.path:
        sys.path.insert(0, _p)

import numpy as np
import ml_dtypes

import concourse.bass as bass
import concourse.bacc as bacc
import concourse.mybir as mybir
import concourse.tile as tile
from concourse import bass_utils
from concourse.masks import make_identity

f32 = mybir.dt.float32
f32r = mybir.dt.float32r
bf16 = mybir.dt.bfloat16

B, S, D, R, E = 4, 2048, 2048, 64, 8
N = B * S                    # 8192 tokens
NCORES = 8
NTOK = N // NCORES           # 1024 tokens per core
SCALING = 64.0 / 16.0
ER = E * R                   # 512
KD = D // 128                # 16 d-chunks
PBLK = 512                   # tokens per pair-block
NPAIR = NTOK // PBLK         # 2
ERCH = ER // 128             # er chunks (4)
NT = PBLK // 128             # tok-chunks per pair (4)
NOCH = D // 512              # output chunks of 512 (4)

_CACHE = {}


def _build():
    if "nc" in _CACHE:
        return _CACHE["nc"]
    nc = bacc.Bacc("TRN2", target_bir_lowering=False, debug=False,
                   num_devices=NCORES)
    xt = nc.dram_tensor("xt", [D, NTOK], f32r, kind="ExternalInput")
    af = nc.dram_tensor("af", [128, KD, ER], bf16, kind="ExternalInput")
    bw = nc.dram_tensor("bw", [128, ERCH, D], bf16, kind="ExternalInput")
    wg = nc.dram_tensor("wg", [128, KD, E], f32r, kind="ExternalInput")
    yo = nc.dram_tensor("yo", [NPAIR, NT, 128, D], bf16, kind="ExternalOutput")
    mstage = nc.dram_tensor("mstage", [NPAIR, NT, E, 128], f32, kind="Internal")

    with tile.TileContext(nc) as tc:
        import contextlib
        ctx = contextlib.ExitStack()
        with ctx:
            singles = ctx.enter_context(tc.tile_pool(name="singles", bufs=1))
            xhip = ctx.enter_context(tc.tile_pool(name="xhip", bufs=2))
            hpool = ctx.enter_context(tc.tile_pool(name="hpool", bufs=2))
            mpool = ctx.enter_context(tc.tile_pool(name="mpool", bufs=2))
            spool = ctx.enter_context(tc.tile_pool(name="spool", bufs=2))
            ypool = ctx.enter_context(tc.tile_pool(name="ypool", bufs=6))
            ps_h = ctx.enter_context(tc.tile_pool(name="ps_h", bufs=1, space="PSUM"))
            ps_lg = ctx.enter_context(tc.tile_pool(name="ps_lg", bufs=1, space="PSUM"))
            ps_tr = ctx.enter_context(tc.tile_pool(name="ps_tr", bufs=1, space="PSUM"))
            ps_y = ctx.enter_context(tc.tile_pool(name="ps_y", bufs=1, space="PSUM"))

            # ---- gate weights + identities ----
            wg_sb = singles.tile([128, KD, E], f32r)
            nc.sync.dma_start(out=wg_sb, in_=wg.ap())
            ident = singles.tile([128, 128], f32)
            make_identity(nc, ident)
            identb = singles.tile([128, 128], bf16)
            make_identity(nc, identb)

            # ---- f32r x pieces for gating+casts, per (pair, k) ----
            xp = [[singles.tile([128, PBLK], f32r, tag=f"xp{p}_{k}",
                                name=f"xp{p}_{k}")
                   for k in range(KD)] for p in range(NPAIR)]
            for k in range(KD):
                eng = nc.sync if k % 2 == 0 else nc.scalar
                eng.dma_start(
                    out=xp[0][k],
                    in_=xt.ap()[128 * k:128 * k + 128, 0:PBLK])

            # gpsimd: adapter A pieces (h stationary), then most of B
            abf_sb = singles.tile([128, KD, ER], bf16)
            for k in range(KD):
                nc.gpsimd.dma_start(out=abf_sb[:, k, :],
                                    in_=af.ap()[:, k, :])
            # gpsimd's idle window (after A, before B is needed) carries
            # the first 4 pair-1 x pieces so xp1 completes ~10us earlier
            for k in range(4):
                nc.gpsimd.dma_start(
                    out=xp[1][k],
                    in_=xt.ap()[128 * k:128 * k + 128, PBLK:NTOK])
            bwr_sb = singles.tile([128, ERCH, D], bf16)
            for oh in range(2):
                for i in range(ERCH):
                    nc.gpsimd.dma_start(
                        out=bwr_sb[:, i, 1024 * oh:1024 * oh + 1024],
                        in_=bw.ap()[:, i, 1024 * oh:1024 * oh + 1024])

            def issue_xp1_and_bwr_tail():
                for k in range(4, KD):
                    eng = nc.sync if k % 2 == 0 else nc.scalar
                    eng.dma_start(
                        out=xp[1][k],
                        in_=xt.ap()[128 * k:128 * k + 128, PBLK:NTOK])

            # ---- PE warm-up while the first x pieces stream in ----
            wup = ps_y.tile([128, 512], f32, tag="y0", name="warmup_ps")
            for w in range(45):
                nc.tensor.matmul(wup[:, 0:128], identb, identb,
                                 start=True, stop=True)

            yrot = [0]

            def ypsum(name):
                # 3-slot rotation: two ps_y banks + the gating bank
                # (idle once pair-1 gating stops) so each chain's
                # bank-release copy is off the next chain's critical path
                tag = ("y0", "y1", "lg")[yrot[0] % 3]
                yrot[0] += 1
                pool = ps_lg if tag == "lg" else ps_y
                return pool.tile([128, 512], f32, tag=tag, name=name)

            def gating(pair):
                # logits^T [E, 512] accumulated over all 16 k-chunks (f32r)
                lg = ps_lg.tile([128, PBLK], f32, tag="lg", name=f"lg{pair}")
                for k in range(KD):
                    nc.tensor.matmul(
                        lg[0:E, :],
                        wg_sb[:, k, :],
                        xp[pair][k],
                        start=(k == 0), stop=(k == KD - 1))
                return lg

            def casts(pair):
                # x -> bf16 for the h matmuls, split over ACT/DVE
                xhi = xhip.tile([128, KD, PBLK], bf16, tag="xhi")
                for k in range(KD):
                    src = xp[pair][k].bitcast(f32)
                    if k % 2 == 0:
                        nc.vector.tensor_copy(xhi[:, k, :], src)
                    else:
                        nc.scalar.copy(xhi[:, k, :], src)
                return xhi

            def h_chunk(pair, xhi, i):
                hp = ps_h.tile([128, PBLK], f32, tag=f"h{i}",
                               name=f"h{pair}_{i}")
                for k in range(KD):
                    nc.tensor.matmul(
                        hp,
                        abf_sb[:, k, 128 * i:128 * i + 128],
                        xhi[:, k, :],
                        start=(k == 0), stop=(k == KD - 1))
                return hp

            def lg_copy(pair, lg):
                lg_sb = spool.tile([E, PBLK], f32, tag="lg_sb")
                nc.scalar.copy(lg_sb, lg[0:E, :])
                return lg_sb

            def tr_logits(pair, lg_sb):
                # PE transpose to natural [tok, e] per 128-token chunk
                eptr = ps_tr.tile([128, 512], f32, tag="tr", name=f"tr{pair}")
                for t in range(NT):
                    nc.tensor.transpose(
                        eptr[:, 8 * t:8 * t + 8],
                        lg_sb[:, 128 * t:128 * t + 128], ident[0:E, 0:E])
                return eptr

            def softmax_mask(pair, eptr):
                # maskval[tok, e] = (logit == max) * prob_sel
                mxs, negs, ses = [], [], []
                for t in range(NT):
                    mx = spool.tile([128, 1], f32, tag=f"mx{t}")
                    nc.vector.reduce_max(out=mx, in_=eptr[:, 8 * t:8 * t + 8],
                                         axis=mybir.AxisListType.X)
                    mxs.append(mx)
                for t in range(NT):
                    negmx = spool.tile([128, 1], f32, tag=f"negmx{t}")
                    nc.vector.tensor_scalar_mul(negmx, mxs[t], -1.0)
                    negs.append(negmx)
                for t in range(NT):
                    es = spool.tile([128, 8], f32, tag=f"es{t}")
                    se = spool.tile([128, 1], f32, tag=f"se{t}")
                    nc.scalar.activation(out=es, in_=eptr[:, 8 * t:8 * t + 8],
                                         func=mybir.ActivationFunctionType.Exp,
                                         bias=negs[t], scale=1.0, accum_out=se)
                    ses.append(se)
                rcps = []
                for t in range(NT):
                    rcp = spool.tile([128, 1], f32, tag=f"rcp{t}")
                    nc.vector.reciprocal(rcp, ses[t])
                    rcps.append(rcp)
                mval4 = spool.tile([128, NT, 8], f32, tag="mval4")
                for t in range(NT):
                    nc.vector.tensor_scalar(
                        out=mval4[:, t, :], in0=eptr[:, 8 * t:8 * t + 8],
                        scalar1=mxs[t], scalar2=rcps[t],
                        op0=mybir.AluOpType.is_equal, op1=mybir.AluOpType.mult)
                return mval4

            def mask_transpose(pair, eptr, mval4):
                # one fused transpose [128 tok, (t e)=32] -> [32, 128]
                nc.tensor.transpose(eptr[0:32, 128:256],
                                    mval4.rearrange("p t e -> p (t e)"), ident)
                mvT4 = mpool.tile([32, 128], f32, tag="mvT4")
                nc.scalar.copy(mvT4, eptr[0:32, 128:256])
                return mvT4

            def mask_expand(pair, mvT4, eng):
                # stage to DRAM, broadcast over the 64 ranks of each expert
                eng.dma_start(out=mstage.ap()[pair], in_=mvT4)
                mexp = []
                for i in range(ERCH):
                    me = mpool.tile([128, PBLK], f32, tag=f"me{i}")
                    mexp.append(me)
                    for half_e in range(2):
                        srcap = bass.AP(
                            tensor=mstage,
                            offset=(pair * (NT * E * 128)
                                    + (2 * i + half_e) * 128),
                            ap=[[0, 64], [E * 128, NT], [1, 128]],
                        )
                        eng.dma_start(
                            out=me[64 * half_e:64 * half_e + 64, :]
                            .rearrange("p (t n) -> p t n", t=NT),
                            in_=srcap)
                return mexp

            def hm_mask_i(pair, hp, me, i):
                hm = hpool.tile([128, PBLK], bf16, tag=f"hm{i}",
                                name=f"hm{pair}_{i}")
                nc.vector.tensor_mul(hm, hp, me)
                return hm

            yq = [0]

            def emit_y(pair, hmT):
                for t in range(NT):
                    ysb = ypool.tile([128, D], bf16, tag="ysb",
                                     name=f"ysb{pair}_{t}")
                    for o in range(NOCH):
                        yp = ypsum(f"yps{pair}_{t}_{o}")
                        for i in range(ERCH):
                            nc.tensor.matmul(
                                yp,
                                hmT[i][:, 128 * t:128 * t + 128],
                                bwr_sb[:, i, 512 * o:512 * o + 512],
                                start=(i == 0), stop=(i == ERCH - 1))
                        o0 = 512 * o
                        nc.scalar.copy(ysb[:, o0:o0 + 256], yp[:, 0:256])
                        nc.vector.tensor_copy(ysb[:, o0 + 256:o0 + 512],
                                              yp[:, 256:512])
                    eng = (nc.sync, nc.scalar, nc.gpsimd)[yq[0] % 3]
                    yq[0] += 1
                    eng.dma_start(out=yo.ap()[pair, t], in_=ysb)

            def pair_front(pair):
                """gating chases x pieces; h chunks follow with the
                epilogue PE ops slotted between so the mask broadcast
                lands while h still streams."""
                xhi = casts(pair)
                lg = gating(pair)
                lgsb = lg_copy(pair, lg)
                hps = [None] * ERCH
                hps[0] = h_chunk(pair, xhi, 0)
                eptr = tr_logits(pair, lgsb)
                hps[1] = h_chunk(pair, xhi, 1)
                mval = softmax_mask(pair, eptr)
                mvT = mask_transpose(pair, eptr, mval)
                hps[2] = h_chunk(pair, xhi, 2)
                mexp = mask_expand(pair, mvT,
                                   nc.scalar if pair == 0 else nc.gpsimd)
                hps[3] = h_chunk(pair, xhi, 3)
                hm = [hm_mask_i(pair, hps[i], mexp[i], i)
                      for i in range(ERCH)]
                return hm

            hm0 = pair_front(0)
            issue_xp1_and_bwr_tail()
            hm1 = pair_front(1)
            emit_y(0, hm0)
            emit_y(1, hm1)

    nc.compile()
    _CACHE["nc"] = nc
    return nc


def _prep_inputs(x, A, Bw, Wg):
    xf = np.ascontiguousarray(np.asarray(x, dtype=np.float32).reshape(N, D))
    xT = np.ascontiguousarray(xf.T)                              # [D, N]
    A_t = np.asarray(A, dtype=np.float32).reshape(ER, D).T       # [D, ER]
    af = np.ascontiguousarray(
        A_t.reshape(KD, 128, ER).transpose(1, 0, 2)).astype(ml_dtypes.bfloat16)
    Bwt = (np.asarray(Bw, dtype=np.float32).transpose(0, 2, 1).reshape(ER, D)
           * SCALING)
    bw = np.ascontiguousarray(
        Bwt.reshape(ERCH, 128, D).transpose(1, 0, 2)).astype(ml_dtypes.bfloat16)
    WgT = np.asarray(Wg, dtype=np.float32).T                     # [D, E]
    wg = np.ascontiguousarray(WgT.reshape(KD, 128, E).transpose(1, 0, 2))
    in_maps = []
    for c in range(NCORES):
        in_maps.append({
            "xt": np.ascontiguousarray(xT[:, c * NTOK:(c + 1) * NTOK]),
            "af": af,
            "bw": bw,
            "wg": wg,
        })
    return in_maps


def _run(x, A, Bw, Wg, trace=False):
    nc = _build()
    in_maps = _prep_inputs(x, A, Bw, Wg)
    res = bass_utils.run_bass_kernel_spmd(
        nc, in_maps, core_ids=list(range(NCORES)), trace=trace)
    y = np.concatenate(
        [np.asarray(res.results[c]["yo"], dtype=np.float32).reshape(NTOK, D)
         for c in range(NCORES)], axis=0)
    return y.reshape(B, S, D), res


def kernel(x, A, Bw, Wg):
    y, _ = _run(x, A, Bw, Wg, trace=False)
    return y


# revision 39
# speedup vs baseline: 1.0542x; 1.0542x over previous
"""MoE low-rank adapters (top-1 routing) Trainium2 kernel.

Math (reference):
  xf = x.reshape(N, D)                 N=8192, D=2048, E=8, R=64
  logits = xf @ Wg.T                   [N, E]
  prob = softmax(logits); gate = argmax(prob); prob_sel = max(prob)
  h = xf @ A[e].T for all e            [N, E*R]
  y = (h * onehot(gate)) @ Bwt         [N, D]
  y *= SCALING * prob_sel

Distribution: data-parallel over tokens, 8 cores x 1024 tokens.

Design (v8 + 3-bank y rotation):
- gating in f32r: 16-matmul chain per 512-token pair into one [8, 512]
  PSUM band (1 cyc/row at free>=256; ~13-bit mantissa keeps every
  argmax identical to fp32 on well-separated logits), then PE
  transposes to natural [tok, e] for the softmax/argmax epilogue.
- h and y in bf16 (full-rate moving streams + FWL weight loads); x is
  cast f32r->bf16 on ACT/DVE, split per k-chunk.
- One dma_start rides one DMA engine (~100-125GB/s), so every large
  transfer is chunked into 128-256KB pieces with >=2KB contiguous
  runs, fanned across the three DMA queues (sync/scalar HWDGE +
  gpsimd SWDGE); aggregate parallelism = outstanding pieces.
- DMA priority: pair-0 x pieces + adapter A first (PE goes dense
  right after warmup so the HAM clock ramps to 2.4GHz); pair-1 x and
  the B weights stream during pair-0 compute; the mask DRAM-broadcast
  rides a quiet queue. y drains per (t, o) [128, 512] chunk over all
  three queues to keep the tail short.
- PE order: warmup, g0, h0 (epilogue PE ops slotted between chunks so
  the mask broadcast lands while h streams), g1, h1, y0, y1.
"""

import sys

for _p in ("/opt/trn_rl_repo",):
    if _p not in sys.path:
        sys.path.insert(0, _p)

import numpy as np
import ml_dtypes

import concourse.bass as bass
import concourse.bacc as bacc
import concourse.mybir as mybir
import concourse.tile as tile
from concourse import bass_utils
from concourse.masks import make_identity

f32 = mybir.dt.float32
f32r = mybir.dt.float32r
bf16 = mybir.dt.bfloat16

B, S, D, R, E = 4, 2048, 2048, 64, 8
N = B * S                    # 8192 tokens
NCORES = 8
NTOK = N // NCORES           # 1024 tokens per core
SCALING = 64.0 / 16.0
ER = E * R                   # 512
KD = D // 128                # 16 d-chunks
PBLK = 512                   # tokens per pair-block
NPAIR = NTOK // PBLK         # 2
ERCH = ER // 128             # er chunks (4)
NT = PBLK // 128             # tok-chunks per pair (4)
NOCH = D // 512              # output chunks of 512 (4)

_CACHE = {}


def _build():
    if "nc" in _CACHE:
        return _CACHE["nc"]
    nc = bacc.Bacc("TRN2", target_bir_lowering=False, debug=False,
                   num_devices=NCORES)
    xt = nc.dram_tensor("xt", [D, NTOK], f32r, kind="ExternalInput")
    af = nc.dram_tensor("af", [128, KD, ER], bf16, kind="ExternalInput")
    bw = nc.dram_tensor("bw", [128, ERCH, D], bf16, kind="ExternalInput")
    wg = nc.dram_tensor("wg", [128, KD, E], f32r, kind="ExternalInput")
    yo = nc.dram_tensor("yo", [NPAIR, NT, 128, D], bf16, kind="ExternalOutput")
    mstage = nc.dram_tensor("mstage", [NPAIR, NT, E, 128], f32, kind="Internal")

    with tile.TileContext(nc) as tc:
        import contextlib
        ctx = contextlib.ExitStack()
        with ctx:
            singles = ctx.enter_context(tc.tile_pool(name="singles", bufs=1))
            xhip = ctx.enter_context(tc.tile_pool(name="xhip", bufs=2))
            hpool = ctx.enter_context(tc.tile_pool(name="hpool", bufs=2))
            mpool = ctx.enter_context(tc.tile_pool(name="mpool", bufs=2))
            spool = ctx.enter_context(tc.tile_pool(name="spool", bufs=2))
            ypool = ctx.enter_context(tc.tile_pool(name="ypool", bufs=6))
            ps_h = ctx.enter_context(tc.tile_pool(name="ps_h", bufs=1, space="PSUM"))
            ps_lg = ctx.enter_context(tc.tile_pool(name="ps_lg", bufs=1, space="PSUM"))
            ps_tr = ctx.enter_context(tc.tile_pool(name="ps_tr", bufs=1, space="PSUM"))
            ps_y = ctx.enter_context(tc.tile_pool(name="ps_y", bufs=1, space="PSUM"))

            # ---- gate weights + identities ----
            wg_sb = singles.tile([128, KD, E], f32r)
            nc.sync.dma_start(out=wg_sb, in_=wg.ap())
            ident = singles.tile([128, 128], f32)
            make_identity(nc, ident)
            identb = singles.tile([128, 128], bf16)
            make_identity(nc, identb)

            # ---- f32r x pieces for gating+casts, per (pair, k) ----
            xp = [[singles.tile([128, PBLK], f32r, tag=f"xp{p}_{k}",
                                name=f"xp{p}_{k}")
                   for k in range(KD)] for p in range(NPAIR)]
            for k in range(KD):
                eng = nc.sync if k % 2 == 0 else nc.scalar
                eng.dma_start(
                    out=xp[0][k],
                    in_=xt.ap()[128 * k:128 * k + 128, 0:PBLK])

            # gpsimd: adapter A pieces (h stationary), then most of B
            abf_sb = singles.tile([128, KD, ER], bf16)
            for k in range(KD):
                nc.gpsimd.dma_start(out=abf_sb[:, k, :],
                                    in_=af.ap()[:, k, :])
            bwr_sb = singles.tile([128, ERCH, D], bf16)
            for i in range(ERCH):
                for oh in range(2):
                    if i == ERCH - 1:
                        continue  # last i-chunk rides sync after xp1
                    nc.gpsimd.dma_start(
                        out=bwr_sb[:, i, 1024 * oh:1024 * oh + 1024],
                        in_=bw.ap()[:, i, 1024 * oh:1024 * oh + 1024])

            def issue_xp1_and_bwr_tail():
                for k in range(KD):
                    eng = nc.sync if k % 2 == 0 else nc.scalar
                    eng.dma_start(
                        out=xp[1][k],
                        in_=xt.ap()[128 * k:128 * k + 128, PBLK:NTOK])
                i = ERCH - 1
                for oh in range(2):
                    nc.gpsimd.dma_start(
                        out=bwr_sb[:, i, 1024 * oh:1024 * oh + 1024],
                        in_=bw.ap()[:, i, 1024 * oh:1024 * oh + 1024])

            # ---- PE warm-up while the first x pieces stream in ----
            wup = ps_y.tile([128, 512], f32, tag="y0", name="warmup_ps")
            for w in range(45):
                nc.tensor.matmul(wup[:, 0:128], identb, identb,
                                 start=True, stop=True)

            yrot = [0]

            def ypsum(name):
                # 3-slot rotation: two ps_y banks + the gating bank
                # (idle once pair-1 gating stops) so each chain's
                # bank-release copy is off the next chain's critical path
                tag = ("y0", "y1", "lg")[yrot[0] % 3]
                yrot[0] += 1
                pool = ps_lg if tag == "lg" else ps_y
                return pool.tile([128, 512], f32, tag=tag, name=name)

            def gating(pair):
                # logits^T [E, 512] accumulated over all 16 k-chunks (f32r)
                lg = ps_lg.tile([128, PBLK], f32, tag="lg", name=f"lg{pair}")
                for k in range(KD):
                    nc.tensor.matmul(
                        lg[0:E, :],
                        wg_sb[:, k, :],
                        xp[pair][k],
                        start=(k == 0), stop=(k == KD - 1))
                return lg

            def casts(pair):
                # x -> bf16 for the h matmuls, split over ACT/DVE
                xhi = xhip.tile([128, KD, PBLK], bf16, tag="xhi")
                for k in range(KD):
                    src = xp[pair][k].bitcast(f32)
                    if k % 2 == 0:
                        nc.vector.tensor_copy(xhi[:, k, :], src)
                    else:
                        nc.scalar.copy(xhi[:, k, :], src)
                return xhi

            def h_chunk(pair, xhi, i):
                hp = ps_h.tile([128, PBLK], f32, tag=f"h{i}",
                               name=f"h{pair}_{i}")
                for k in range(KD):
                    nc.tensor.matmul(
                        hp,
                        abf_sb[:, k, 128 * i:128 * i + 128],
                        xhi[:, k, :],
                        start=(k == 0), stop=(k == KD - 1))
                return hp

            def lg_copy(pair, lg):
                lg_sb = spool.tile([E, PBLK], f32, tag="lg_sb")
                nc.scalar.copy(lg_sb, lg[0:E, :])
                return lg_sb

            def tr_logits(pair, lg_sb):
                # PE transpose to natural [tok, e] per 128-token chunk
                eptr = ps_tr.tile([128, 512], f32, tag="tr", name=f"tr{pair}")
                for t in range(NT):
                    nc.tensor.transpose(
                        eptr[:, 8 * t:8 * t + 8],
                        lg_sb[:, 128 * t:128 * t + 128], ident[0:E, 0:E])
                return eptr

            def softmax_mask(pair, eptr):
                # maskval[tok, e] = (logit == max) * prob_sel
                mxs, negs, ses = [], [], []
                for t in range(NT):
                    mx = spool.tile([128, 1], f32, tag=f"mx{t}")
                    nc.vector.reduce_max(out=mx, in_=eptr[:, 8 * t:8 * t + 8],
                                         axis=mybir.AxisListType.X)
                    mxs.append(mx)
                for t in range(NT):
                    negmx = spool.tile([128, 1], f32, tag=f"negmx{t}")
                    nc.vector.tensor_scalar_mul(negmx, mxs[t], -1.0)
                    negs.append(negmx)
                for t in range(NT):
                    es = spool.tile([128, 8], f32, tag=f"es{t}")
                    se = spool.tile([128, 1], f32, tag=f"se{t}")
                    nc.scalar.activation(out=es, in_=eptr[:, 8 * t:8 * t + 8],
                                         func=mybir.ActivationFunctionType.Exp,
                                         bias=negs[t], scale=1.0, accum_out=se)
                    ses.append(se)
                rcps = []
                for t in range(NT):
                    rcp = spool.tile([128, 1], f32, tag=f"rcp{t}")
                    nc.vector.reciprocal(rcp, ses[t])
                    rcps.append(rcp)
                mval4 = spool.tile([128, NT, 8], f32, tag="mval4")
                for t in range(NT):
                    nc.vector.tensor_scalar(
                        out=mval4[:, t, :], in0=eptr[:, 8 * t:8 * t + 8],
                        scalar1=mxs[t], scalar2=rcps[t],
                        op0=mybir.AluOpType.is_equal, op1=mybir.AluOpType.mult)
                return mval4

            def mask_transpose(pair, eptr, mval4):
                # one fused transpose [128 tok, (t e)=32] -> [32, 128]
                nc.tensor.transpose(eptr[0:32, 128:256],
                                    mval4.rearrange("p t e -> p (t e)"), ident)
                mvT4 = mpool.tile([32, 128], f32, tag="mvT4")
                nc.scalar.copy(mvT4, eptr[0:32, 128:256])
                return mvT4

            def mask_expand(pair, mvT4, eng):
                # stage to DRAM, broadcast over the 64 ranks of each expert
                eng.dma_start(out=mstage.ap()[pair], in_=mvT4)
                mexp = []
                for i in range(ERCH):
                    me = mpool.tile([128, PBLK], f32, tag=f"me{i}")
                    mexp.append(me)
                    for half_e in range(2):
                        srcap = bass.AP(
                            tensor=mstage,
                            offset=(pair * (NT * E * 128)
                                    + (2 * i + half_e) * 128),
                            ap=[[0, 64], [E * 128, NT], [1, 128]],
                        )
                        eng.dma_start(
                            out=me[64 * half_e:64 * half_e + 64, :]
                            .rearrange("p (t n) -> p t n", t=NT),
                            in_=srcap)
                return mexp

            def hm_mask_i(pair, hp, me, i):
                hm = hpool.tile([128, PBLK], bf16, tag=f"hm{i}",
                                name=f"hm{pair}_{i}")
                nc.vector.tensor_mul(hm, hp, me)
                return hm

            yq = [0]

            def emit_y(pair, hmT):
                for t in range(NT):
                    ysb = ypool.tile([128, D], bf16, tag="ysb",
                                     name=f"ysb{pair}_{t}")
                    for o in range(NOCH):
                        yp = ypsum(f"yps{pair}_{t}_{o}")
                        for i in range(ERCH):
                            nc.tensor.matmul(
                                yp,
                                hmT[i][:, 128 * t:128 * t + 128],
                                bwr_sb[:, i, 512 * o:512 * o + 512],
                                start=(i == 0), stop=(i == ERCH - 1))
                        o0 = 512 * o
                        nc.scalar.copy(ysb[:, o0:o0 + 256], yp[:, 0:256])
                        nc.vector.tensor_copy(ysb[:, o0 + 256:o0 + 512],
                                              yp[:, 256:512])
                    eng = (nc.sync, nc.scalar, nc.gpsimd)[yq[0] % 3]
                    yq[0] += 1
                    eng.dma_start(out=yo.ap()[pair, t], in_=ysb)

            def pair_front(pair):
                """gating chases x pieces; h chunks follow with the
                epilogue PE ops slotted between so the mask broadcast
                lands while h still streams."""
                xhi = casts(pair)
                lg = gating(pair)
                lgsb = lg_copy(pair, lg)
                hps = [None] * ERCH
                hps[0] = h_chunk(pair, xhi, 0)
                eptr = tr_logits(pair, lgsb)
                hps[1] = h_chunk(pair, xhi, 1)
                mval = softmax_mask(pair, eptr)
                mvT = mask_transpose(pair, eptr, mval)
                hps[2] = h_chunk(pair, xhi, 2)
                mexp = mask_expand(pair, mvT,
                                   nc.scalar if pair == 0 else nc.gpsimd)
                hps[3] = h_chunk(pair, xhi, 3)
                hm = [hm_mask_i(pair, hps[i], mexp[i], i)
                      for i in range(ERCH)]
                return hm

            hm0 = pair_front(0)
            issue_xp1_and_bwr_tail()
            hm1 = pair_front(1)
            emit_y(0, hm0)
            emit_y(1, hm1)

    nc.compile()
    _CACHE["nc"] = nc
    return nc


def _prep_inputs(x, A, Bw, Wg):
    xf = np.ascontiguousarray(np.asarray(x, dtype=np.float32).reshape(N, D))
    xT = np.ascontiguousarray(xf.T)                              # [D, N]
    A_t = np.asarray(A, dtype=np.float32).reshape(ER, D).T       # [D, ER]
    af = np.ascontiguousarray(
        A_t.reshape(KD, 128, ER).transpose(1, 0, 2)).astype(ml_dtypes.bfloat16)
    Bwt = (np.asarray(Bw, dtype=np.float32).transpose(0, 2, 1).reshape(ER, D)
           * SCALING)
    bw = np.ascontiguousarray(
        Bwt.reshape(ERCH, 128, D).transpose(1, 0, 2)).astype(ml_dtypes.bfloat16)
    WgT = np.asarray(Wg, dtype=np.float32).T                     # [D, E]
    wg = np.ascontiguousarray(WgT.reshape(KD, 128, E).transpose(1, 0, 2))
    in_maps = []
    for c in range(NCORES):
        in_maps.append({
            "xt": np.ascontiguousarray(xT[:, c * NTOK:(c + 1) * NTOK]),
            "af": af,
            "bw": bw,
            "wg": wg,
        })
    return in_maps


def _run(x, A, Bw, Wg, trace=False):
    nc = _build()
    in_maps = _prep_inputs(x, A, Bw, Wg)
    res = bass_utils.run_bass_kernel_spmd(
        nc, in_maps, core_ids=list(range(NCORES)), trace=trace)
    y = np.concatenate(
        [np.asarray(res.results[c]["yo"], dtype=np.float32).reshape(NTOK, D)
         for c in range(NCORES)], axis=0)
    return y.reshape(B, S, D), res


def kernel(x, A, Bw, Wg):
    y, _ = _run(x, A, Bw, Wg, trace=False)
    return y
